# revision 15
# baseline (speedup 1.0000x reference)
"""Multi-head causal attention (B=2, S=2048, E=1024, H=16, D=64) on 8 TRN2 cores.

Sharding: core c -> batch b = c // 4, head group g = c % 4 (4 heads each).
Each core computes q/k/v projections + RoPE + causal attention + its rows of
the Wo projection for its (batch, head-group); the host sums the 4 row-parallel
Wo partials per batch (the unshard step of row-parallel output projection).

Device layout notes:
  - x is passed pre-transposed per batch: xT [E, S] so the PE can contract
    over E (partition dim) for the projections.
  - q/k are computed transposed (qT/kT [64, S]) with head-pair fused weights
    so one [128, 512] PSUM tile holds [q_x1; q_x2; k_x1; k_x2] rows, where
    x1/x2 are the RoPE even/odd pair halves (weight columns pre-permuted on
    host so rotate-half applies).
  - scores are computed transposed, sT [k, q] = kT.T @ qT; softmax runs over
    the partition dim via an appended ones-column in the AV matmul (Z row).
    No max-subtraction: scores ~ N(0,1), exp is safe in fp32.
  - AV computes attnT [d, q]; Wo projection contracts head-dim chunks of
    attnT against Wo rows (fp16), accumulating out [s, e] tiles in PSUM.
"""

import sys

if "/opt/trn_rl_repo" not in sys.path:
    sys.path.insert(0, "/opt/trn_rl_repo")

import numpy as np

import concourse.bass as bass
import concourse.tile as tile
from concourse import bacc, mybir
from concourse.bass_utils import run_bass_kernel_spmd

B, S, E, H, D = 2, 2048, 1024, 16, 64
HPC = 4  # heads per core
NCORES = 8
SB = 512  # q/s block width
NSB = S // SB  # 4
KT = 128  # k tile (partition chunk of the sequence)
NKT = S // KT  # 16
ECH = E // 128  # 8 contraction chunks for the projections

f32 = mybir.dt.float32
f16 = mybir.dt.float16
bf16 = mybir.dt.bfloat16

ROPE_BASE = 10000.0


def build_nc():
    nc = bacc.Bacc(
        "TRN2", target_bir_lowering=False, debug=False, enable_asserts=False
    )

    xT_d = nc.dram_tensor("xT", [E, S], f16, kind="ExternalInput")
    wqk_d = nc.dram_tensor("wqk", [E, HPC, 128], f16, kind="ExternalInput")
    wv_d = nc.dram_tensor("wv", [E, HPC * D], f16, kind="ExternalInput")
    wo_d = nc.dram_tensor("wo", [HPC * D, E], f16, kind="ExternalInput")
    cos_d = nc.dram_tensor("cos2", [128, S], f32, kind="ExternalInput")
    sin_d = nc.dram_tensor("sin2", [128, S], f32, kind="ExternalInput")
    mask_d = nc.dram_tensor("maskb", [128, 4, SB], f16, kind="ExternalInput")
    tri_d = nc.dram_tensor("tri", [128, 128], f16, kind="ExternalInput")
    out_d = nc.dram_tensor("out", [S, E], f16, kind="ExternalOutput")

    with tile.TileContext(nc) as tc:
        with (
            tc.tile_pool(name="const", bufs=1) as constp,
            tc.tile_pool(name="qk", bufs=1) as qkp,
            tc.tile_pool(name="vb", bufs=1) as vbp,
            tc.tile_pool(name="at", bufs=1) as atp,
            tc.tile_pool(name="st", bufs=10) as stp,
            tc.tile_pool(name="tmp", bufs=4) as tmpp,
            tc.tile_pool(name="mm", bufs=5, space="PSUM") as mmp,
            tc.tile_pool(name="acc", bufs=3, space="PSUM") as accp,
        ):
            # ---- constant tiles (DMAs issued per s-block, in consumption
            # order, so the first projection matmuls start within a few us) --
            xT_ap = xT_d.ap().rearrange("(eo p) s -> eo p s", p=128)
            xT = [
                constp.tile([128, S], f16, tag=f"xT{e}", name=f"xT{e}")
                for e in range(ECH)
            ]
            wqk = constp.tile([128, ECH, HPC, 128], f16, tag="wqk")
            nc.sync.dma_start(
                out=wqk, in_=wqk_d.ap().rearrange("(eo p) h m -> p eo h m", p=128)
            )
            wv = constp.tile([128, ECH, HPC * D], f16, tag="wv")
            nc.sync.dma_start(
                out=wv, in_=wv_d.ap().rearrange("(eo p) m -> p eo m", p=128)
            )
            cos2 = constp.tile([128, S], f32, tag="cos2")
            sin2 = constp.tile([128, S], f32, tag="sin2")
            maskb = constp.tile([128, 4, SB], f16, tag="maskb")
            tri = constp.tile([128, 128], f16, tag="tri")
            wo = constp.tile([128, 2, E], f16, tag="wo")

            def emit_loads(sb):
                cs = slice(sb * SB, (sb + 1) * SB)
                for e in range(ECH):
                    nc.sync.dma_start(out=xT[e][:, cs], in_=xT_ap[e][:, cs])
                nc.sync.dma_start(out=cos2[:, cs], in_=cos_d.ap()[:, cs])
                nc.sync.dma_start(out=sin2[:, cs], in_=sin_d.ap()[:, cs])
                if sb == 0:
                    nc.sync.dma_start(out=maskb, in_=mask_d.ap())
                    nc.sync.dma_start(out=tri, in_=tri_d.ap())
                if sb == 1:
                    nc.sync.dma_start(
                        out=wo, in_=wo_d.ap().rearrange("(c p) e -> p c e", p=128)
                    )

            # qq[p] rows: qT of head 2p on partitions 0-63, head 2p+1 on 64-127
            # (kk[p] likewise) so each head's scores matmul operands share a
            # partition base. psum rows per head: [q_x1; q_x2; k_x1; k_x2].
            qq = [
                qkp.tile([128, S], f16, tag=f"qq{p}", name=f"qq{p}")
                for p in range(2)
            ]
            kk = [
                qkp.tile([128, S], f16, tag=f"kk{p}", name=f"kk{p}")
                for p in range(2)
            ]
            swap_src = [32, 0, 96, 64]

            def emit_qk_proj(sb):
                cs = slice(sb * SB, (sb + 1) * SB)
                for h in range(HPC):
                    p, half = h // 2, (h % 2) * 64
                    ps = mmp.tile([128, SB], f32, tag="mm", name="ps")
                    for e in range(ECH):
                        nc.tensor.matmul(
                            out=ps,
                            lhsT=wqk[:, e, h, :],
                            rhs=xT[e][:, cs],
                            start=(e == 0),
                            stop=(e == ECH - 1),
                        )
                    rs = tmpp.tile([128, SB], f32, tag="rs", name="rs")
                    nc.scalar.copy(out=rs, in_=ps)
                    t1 = tmpp.tile([128, SB], f32, tag="t1", name="t1")
                    t2 = tmpp.tile([128, SB], f32, tag="t2", name="t2")
                    nc.vector.tensor_mul(t1, rs, cos2[:, cs])
                    for g in range(4):
                        # sin2 rows are laid out so in0/in1 share a base
                        # partition (walrus SB+SB constraint)
                        srow = swap_src[g]
                        nc.vector.tensor_mul(
                            t2[g * 32 : (g + 1) * 32, :],
                            rs[srow : srow + 32, :],
                            sin2[srow : srow + 32, cs],
                        )
                    nc.vector.tensor_add(
                        qq[p][half : half + 64, cs], t1[0:64, :], t2[0:64, :]
                    )
                    nc.vector.tensor_add(
                        kk[p][half : half + 64, cs], t1[64:128, :], t2[64:128, :]
                    )

            # v_big free layout per k-chunk: 4 heads x [v_h (64) | one (1)]
            v_big = vbp.tile([128, NKT, HPC * 65], f16, tag="vbig")
            ones_cols = v_big.rearrange("p n (h m) -> p n h m", h=HPC)[
                :, :, :, 64:65
            ]
            nc.vector.memset(ones_cols, 1.0)

            def emit_v_proj(sb):
                for kc in range(4 * sb, 4 * sb + 4):
                    vps = mmp.tile([128, HPC * D], f32, tag="mm", name="vps")
                    for e in range(ECH):
                        nc.tensor.matmul(
                            out=vps,
                            lhsT=xT[e][:, kc * KT : (kc + 1) * KT],
                            rhs=wv[:, e, :],
                            start=(e == 0),
                            stop=(e == ECH - 1),
                        )
                    nc.vector.tensor_copy(
                        out=v_big.rearrange("p n (h m) -> p n h m", h=HPC)[
                            :, kc, :, 0:64
                        ],
                        in_=vps.rearrange("p (h m) -> p h m", h=HPC),
                    )

            # ---- phase C: attention per (q block, head pair) --------------------
            # attnT tiles: at8[c][qb] rows = hd chunk c (2 heads x 64), cols = q
            # Heads 2p / 2p+1 sit at partition bases 0 / 64 of qq[p]/kk[p], so
            # their K=64 scores matmuls land in disjoint PE row groups and run
            # concurrently (row tiling via auto tile_position).
            at8 = {}
            for c in range(2):
                for qb in range(NSB):
                    at8[(c, qb)] = atp.tile(
                        [128, SB], f16, tag=f"at{c}_{qb}", name=f"at{c}_{qb}"
                    )

            def emit_attn(qb):
                qs = slice(qb * SB, (qb + 1) * SB)
                n_k = 4 * (qb + 1)
                for p in range(2):
                    avs = [
                        accp.tile([128, SB], f32, tag="acc", name=f"av{i}")
                        for i in range(2)
                    ]
                    # Software pipeline: emit the AV matmul for chunk kt only
                    # LAG steps after its scores matmul, so the PE (strict
                    # in-order queue) never head-of-line blocks on the ACT exp.
                    LAG = 3
                    sts_buf = {}
                    for step in range(n_k + LAG):
                        if step < n_k:
                            kt = step
                            j = kt - 4 * qb
                            kts = slice(kt * KT, (kt + 1) * KT)
                            pss, sts = [], []
                            for i in range(2):
                                half = i * 64
                                ps = mmp.tile([128, SB], f32, tag="mm", name="ps")
                                nc.tensor.matmul(
                                    out=ps,
                                    lhsT=kk[p][half : half + 64, kts],
                                    rhs=qq[p][half : half + 64, qs],
                                    start=True,
                                    stop=(j < 0),
                                )
                                if j >= 0:
                                    # causal mask: add -240*max(0, r+128j-c)
                                    # (tri.T @ maskb_j); exp(0.125*x) -> 0
                                    nc.tensor.matmul(
                                        out=ps,
                                        lhsT=tri,
                                        rhs=maskb[:, j, :],
                                        start=False,
                                        stop=True,
                                    )
                                pss.append(ps)
                            for i in range(2):
                                st_t = stp.tile(
                                    [128, SB], f16, tag="st", name="st_t"
                                )
                                nc.scalar.activation(
                                    out=st_t,
                                    in_=pss[i],
                                    func=mybir.ActivationFunctionType.Exp,
                                    scale=0.125,
                                )
                                sts.append(st_t)
                            sts_buf[kt] = sts
                        if step >= LAG:
                            kt = step - LAG
                            sts = sts_buf.pop(kt)
                            for i in range(2):
                                h = 2 * p + i
                                nc.tensor.matmul(
                                    out=avs[i][0:65, :],
                                    lhsT=v_big[:, kt, h * 65 : (h + 1) * 65],
                                    rhs=sts[i],
                                    start=(kt == 0),
                                    stop=(kt == n_k - 1),
                                )
                    # normalize: attnT = av[0:64] / Z  (Z = av row 64)
                    for i in range(2):
                        h = 2 * p + i
                        r = tmpp.tile([1, SB], f32, tag="r", name="r")
                        nc.vector.reciprocal(out=r, in_=avs[i][64:65, :])
                        zb = tmpp.tile([64, SB], f32, tag="zb", name="zb")
                        nc.gpsimd.partition_broadcast(zb, r)
                        c, half = h // 2, (h % 2) * 64
                        nc.vector.tensor_mul(
                            at8[(c, qb)][half : half + 64, :], avs[i][0:64, :], zb
                        )

            # ---- phase D: output projection (row-parallel partial) -------------
            def emit_out_proj(qb):
                for stl in range(4):
                    rows = qb * SB + stl * KT
                    for eb in range(2):
                        pw = mmp.tile([128, SB], f32, tag="mm", name="pw")
                        for c in range(2):
                            nc.tensor.matmul(
                                out=pw,
                                lhsT=at8[(c, qb)][:, stl * KT : (stl + 1) * KT],
                                rhs=wo[:, c, eb * SB : (eb + 1) * SB],
                                start=(c == 0),
                                stop=(c == 1),
                            )
                        ot = stp.tile([128, SB], f16, tag="ot", name="ot", bufs=3)
                        nc.vector.tensor_copy(out=ot, in_=pw)
                        nc.sync.dma_start(
                            out=out_d.ap()[rows : rows + KT, eb * SB : (eb + 1) * SB],
                            in_=ot,
                        )

            # ---- emission schedule: pipeline loads/proj with attention ----------
            emit_loads(0)
            emit_qk_proj(0)
            emit_v_proj(0)
            emit_loads(1)
            emit_qk_proj(1)
            emit_v_proj(1)
            emit_attn(0)
            emit_loads(2)
            emit_qk_proj(2)
            emit_v_proj(2)
            emit_attn(1)
            emit_loads(3)
            emit_qk_proj(3)
            emit_v_proj(3)
            emit_out_proj(0)
            emit_attn(2)
            emit_out_proj(1)
            emit_attn(3)
            emit_out_proj(2)
            emit_out_proj(3)

    nc.compile()
    return nc


def build_in_maps(x, Wq, Wk, Wv, Wo):
    x = np.asarray(x, np.float32)
    Wq = np.asarray(Wq, np.float32)
    Wk = np.asarray(Wk, np.float32)
    Wv = np.asarray(Wv, np.float32)
    Wo = np.asarray(Wo, np.float32)

    # RoPE tables in rotate-half layout ([32] pair-frequencies, duplicated)
    inv = 1.0 / (ROPE_BASE ** (np.arange(0, D, 2, dtype=np.float64) / D))  # [32]
    ang = inv[:, None] * np.arange(S, dtype=np.float64)[None, :]  # [32, S]
    cos_t = np.cos(ang).astype(np.float32)
    sin_t = np.sin(ang).astype(np.float32)
    cos2 = np.concatenate([cos_t, cos_t, cos_t, cos_t], 0)  # [128, S]
    sin2 = np.concatenate([sin_t, -sin_t, sin_t, -sin_t], 0)  # [128, S] (rows at swap-source positions)

    # Causal mask matmul operands: accumulating tri.T @ maskb_j into the
    # scores psum adds -240*max(0, r + 128j - c), which the exp flushes to 0
    # exactly on the masked (k > q) region.
    tt = np.arange(128)[:, None]
    cc = np.arange(SB)[None, :]
    maskb = np.ascontiguousarray(
        np.stack([(cc < tt + j * KT) for j in range(4)], axis=1)
    ).astype(np.float16)  # [128, 4, SB]
    rr = np.arange(128)[None, :]
    tri = (-240.0 * (tt <= rr)).astype(np.float16)  # [t, r]

    # weight column permutation: even pair-elements then odd (rotate-half)
    perm = np.concatenate([np.arange(0, D, 2), np.arange(1, D, 2)])

    in_maps = []
    for core in range(NCORES):
        b, g = core // HPC, core % HPC
        wqk = np.empty((E, HPC, 128), np.float32)
        for i in range(HPC):
            h = g * HPC + i
            wqk[:, i, 0:64] = Wq[:, h * D : (h + 1) * D][:, perm]
            wqk[:, i, 64:128] = Wk[:, h * D : (h + 1) * D][:, perm]
        in_maps.append(
            {
                "xT": np.ascontiguousarray(x[b].T).astype(np.float16),
                "wqk": wqk.astype(np.float16),
                "wv": np.ascontiguousarray(
                    Wv[:, g * HPC * D : (g + 1) * HPC * D]
                ).astype(np.float16),
                "wo": np.ascontiguousarray(
                    Wo[g * HPC * D : (g + 1) * HPC * D, :]
                ).astype(np.float16),
                "cos2": cos2,
                "sin2": sin2,
                "maskb": maskb,
                "tri": tri,
            }
        )
    return in_maps


def gather_output(results):
    outs = [np.asarray(r["out"], np.float32) for r in results]
    return np.stack(
        [outs[0] + outs[1] + outs[2] + outs[3], outs[4] + outs[5] + outs[6] + outs[7]],
        axis=0,
    )


_NC_CACHE = {}


def kernel(x, Wq, Wk, Wv, Wo):
    in_maps = build_in_maps(x, Wq, Wk, Wv, Wo)
    if "nc" not in _NC_CACHE:
        _NC_CACHE["nc"] = build_nc()
    res = run_bass_kernel_spmd(_NC_CACHE["nc"], in_maps, core_ids=list(range(NCORES)))
    return gather_output(res.results)


# revision 16
# speedup vs baseline: 2.7889x; 2.7889x over previous
"""Multi-head causal attention (B=2, S=2048, E=1024, H=16, D=64) on 8 TRN2 cores.

Sharding: core c -> batch b = c // 4, head group g = c % 4 (4 heads each).
Each core computes q/k/v projections + RoPE + causal attention + its rows of
the Wo projection for its (batch, head-group); the host sums the 4 row-parallel
Wo partials per batch (the unshard step of row-parallel output projection).

Device layout notes:
  - x is passed pre-transposed per batch: xT [E, S] so the PE can contract
    over E (partition dim) for the projections.
  - q/k are computed transposed (qT/kT [64, S]) with head-pair fused weights
    so one [128, 512] PSUM tile holds [q_x1; q_x2; k_x1; k_x2] rows, where
    x1/x2 are the RoPE even/odd pair halves (weight columns pre-permuted on
    host so rotate-half applies).
  - scores are computed transposed, sT [k, q] = kT.T @ qT; softmax runs over
    the partition dim via an appended ones-column in the AV matmul (Z row).
    No max-subtraction: scores ~ N(0,1), exp is safe in fp32.
  - AV computes attnT [d, q]; Wo projection contracts head-dim chunks of
    attnT against Wo rows (fp16), accumulating out [s, e] tiles in PSUM.
"""

import sys

if "/opt/trn_rl_repo" not in sys.path:
    sys.path.insert(0, "/opt/trn_rl_repo")

import numpy as np

import concourse.bass as bass
import concourse.tile as tile
from concourse import bacc, mybir
from concourse.bass_utils import run_bass_kernel_spmd

B, S, E, H, D = 2, 2048, 1024, 16, 64
HPC = 4  # heads per core
NCORES = 8
SB = 512  # q/s block width
NSB = S // SB  # 4
KT = 128  # k tile (partition chunk of the sequence)
NKT = S // KT  # 16
ECH = E // 128  # 8 contraction chunks for the projections

f32 = mybir.dt.float32
f16 = mybir.dt.float16
bf16 = mybir.dt.bfloat16

ROPE_BASE = 10000.0


def build_nc(unroll=1):
    nc = bacc.Bacc(
        "TRN2", target_bir_lowering=False, debug=False, enable_asserts=False
    )

    xT_d = nc.dram_tensor("xT", [E, S], f16, kind="ExternalInput")
    wqk_d = nc.dram_tensor("wqk", [E, HPC, 128], f16, kind="ExternalInput")
    wv_d = nc.dram_tensor("wv", [E, HPC * D], f16, kind="ExternalInput")
    wo_d = nc.dram_tensor("wo", [HPC * D, E], f16, kind="ExternalInput")
    cos_d = nc.dram_tensor("cos2", [128, S], f32, kind="ExternalInput")
    sin_d = nc.dram_tensor("sin2", [128, S], f32, kind="ExternalInput")
    mask_d = nc.dram_tensor("maskb", [128, 4, SB], f16, kind="ExternalInput")
    tri_d = nc.dram_tensor("tri", [128, 128], f16, kind="ExternalInput")
    out_d = nc.dram_tensor("out", [S, E], f16, kind="ExternalOutput")

    with tile.TileContext(nc) as tc:
        with (
            tc.tile_pool(name="const", bufs=1) as constp,
            tc.tile_pool(name="qk", bufs=1) as qkp,
            tc.tile_pool(name="vb", bufs=1) as vbp,
            tc.tile_pool(name="at", bufs=1) as atp,
            tc.tile_pool(name="st", bufs=10) as stp,
            tc.tile_pool(name="tmp", bufs=4) as tmpp,
            tc.tile_pool(name="mm", bufs=5, space="PSUM") as mmp,
            tc.tile_pool(name="acc", bufs=3, space="PSUM") as accp,
        ):
            # ---- constant tiles (DMAs issued per s-block, in consumption
            # order, so the first projection matmuls start within a few us) --
            xT_ap = xT_d.ap().rearrange("(eo p) s -> eo p s", p=128)
            xT = [
                constp.tile([128, S], f16, tag=f"xT{e}", name=f"xT{e}")
                for e in range(ECH)
            ]
            wqk = constp.tile([128, ECH, HPC, 128], f16, tag="wqk")
            nc.sync.dma_start(
                out=wqk, in_=wqk_d.ap().rearrange("(eo p) h m -> p eo h m", p=128)
            )
            wv = constp.tile([128, ECH, HPC * D], f16, tag="wv")
            nc.sync.dma_start(
                out=wv, in_=wv_d.ap().rearrange("(eo p) m -> p eo m", p=128)
            )
            cos2 = constp.tile([128, S], f32, tag="cos2")
            sin2 = constp.tile([128, S], f32, tag="sin2")
            maskb = constp.tile([128, 4, SB], f16, tag="maskb")
            tri = constp.tile([128, 128], f16, tag="tri")
            wo = constp.tile([128, 2, E], f16, tag="wo")

            def emit_loads(sb):
                cs = slice(sb * SB, (sb + 1) * SB)
                for e in range(ECH):
                    nc.sync.dma_start(out=xT[e][:, cs], in_=xT_ap[e][:, cs])
                nc.sync.dma_start(out=cos2[:, cs], in_=cos_d.ap()[:, cs])
                nc.sync.dma_start(out=sin2[:, cs], in_=sin_d.ap()[:, cs])
                if sb == 0:
                    nc.sync.dma_start(out=maskb, in_=mask_d.ap())
                    nc.sync.dma_start(out=tri, in_=tri_d.ap())
                if sb == 1:
                    nc.sync.dma_start(
                        out=wo, in_=wo_d.ap().rearrange("(c p) e -> p c e", p=128)
                    )

            # qq[p] rows: qT of head 2p on partitions 0-63, head 2p+1 on 64-127
            # (kk[p] likewise) so each head's scores matmul operands share a
            # partition base. psum rows per head: [q_x1; q_x2; k_x1; k_x2].
            qq = [
                qkp.tile([128, S], f16, tag=f"qq{p}", name=f"qq{p}")
                for p in range(2)
            ]
            kk = [
                qkp.tile([128, S], f16, tag=f"kk{p}", name=f"kk{p}")
                for p in range(2)
            ]
            swap_src = [32, 0, 96, 64]

            def emit_qk_proj(sb):
                cs = slice(sb * SB, (sb + 1) * SB)
                for h in range(HPC):
                    p, half = h // 2, (h % 2) * 64
                    ps = mmp.tile([128, SB], f32, tag="mm", name="ps")
                    for e in range(ECH):
                        nc.tensor.matmul(
                            out=ps,
                            lhsT=wqk[:, e, h, :],
                            rhs=xT[e][:, cs],
                            start=(e == 0),
                            stop=(e == ECH - 1),
                        )
                    rs = tmpp.tile([128, SB], f32, tag="rs", name="rs")
                    nc.scalar.copy(out=rs, in_=ps)
                    t1 = tmpp.tile([128, SB], f32, tag="t1", name="t1")
                    t2 = tmpp.tile([128, SB], f32, tag="t2", name="t2")
                    nc.vector.tensor_mul(t1, rs, cos2[:, cs])
                    for g in range(4):
                        # sin2 rows are laid out so in0/in1 share a base
                        # partition (walrus SB+SB constraint)
                        srow = swap_src[g]
                        nc.vector.tensor_mul(
                            t2[g * 32 : (g + 1) * 32, :],
                            rs[srow : srow + 32, :],
                            sin2[srow : srow + 32, cs],
                        )
                    nc.vector.tensor_add(
                        qq[p][half : half + 64, cs], t1[0:64, :], t2[0:64, :]
                    )
                    nc.vector.tensor_add(
                        kk[p][half : half + 64, cs], t1[64:128, :], t2[64:128, :]
                    )

            # v_big free layout per k-chunk: 4 heads x [v_h (64) | one (1)]
            v_big = vbp.tile([128, NKT, HPC * 65], f16, tag="vbig")
            ones_cols = v_big.rearrange("p n (h m) -> p n h m", h=HPC)[
                :, :, :, 64:65
            ]
            nc.vector.memset(ones_cols, 1.0)

            def emit_v_proj(sb):
                for kc in range(4 * sb, 4 * sb + 4):
                    vps = mmp.tile([128, HPC * D], f32, tag="mm", name="vps")
                    for e in range(ECH):
                        nc.tensor.matmul(
                            out=vps,
                            lhsT=xT[e][:, kc * KT : (kc + 1) * KT],
                            rhs=wv[:, e, :],
                            start=(e == 0),
                            stop=(e == ECH - 1),
                        )
                    nc.vector.tensor_copy(
                        out=v_big.rearrange("p n (h m) -> p n h m", h=HPC)[
                            :, kc, :, 0:64
                        ],
                        in_=vps.rearrange("p (h m) -> p h m", h=HPC),
                    )

            # ---- phase C: attention per (q block, head pair) --------------------
            # attnT tiles: at8[c][qb] rows = hd chunk c (2 heads x 64), cols = q
            # Heads 2p / 2p+1 sit at partition bases 0 / 64 of qq[p]/kk[p], so
            # their K=64 scores matmuls land in disjoint PE row groups and run
            # concurrently (row tiling via auto tile_position).
            at8 = {}
            for c in range(2):
                for qb in range(NSB):
                    at8[(c, qb)] = atp.tile(
                        [128, SB], f16, tag=f"at{c}_{qb}", name=f"at{c}_{qb}"
                    )

            def emit_attn(qb):
                qs = slice(qb * SB, (qb + 1) * SB)
                n_k = 4 * (qb + 1)
                for p in range(2):
                    avs = [
                        accp.tile([128, SB], f32, tag="acc", name=f"av{i}")
                        for i in range(2)
                    ]
                    # Software pipeline: emit the AV matmul for chunk kt only
                    # LAG steps after its scores matmul, so the PE (strict
                    # in-order queue) never head-of-line blocks on the ACT exp.
                    LAG = 3
                    sts_buf = {}
                    for step in range(n_k + LAG):
                        if step < n_k:
                            kt = step
                            j = kt - 4 * qb
                            kts = slice(kt * KT, (kt + 1) * KT)
                            pss, sts = [], []
                            for i in range(2):
                                half = i * 64
                                ps = mmp.tile([128, SB], f32, tag="mm", name="ps")
                                nc.tensor.matmul(
                                    out=ps,
                                    lhsT=kk[p][half : half + 64, kts],
                                    rhs=qq[p][half : half + 64, qs],
                                    start=True,
                                    stop=(j < 0),
                                )
                                if j >= 0:
                                    # causal mask: add -240*max(0, r+128j-c)
                                    # (tri.T @ maskb_j); exp(0.125*x) -> 0
                                    nc.tensor.matmul(
                                        out=ps,
                                        lhsT=tri,
                                        rhs=maskb[:, j, :],
                                        start=False,
                                        stop=True,
                                    )
                                pss.append(ps)
                            for i in range(2):
                                st_t = stp.tile(
                                    [128, SB], f16, tag="st", name="st_t"
                                )
                                nc.scalar.activation(
                                    out=st_t,
                                    in_=pss[i],
                                    func=mybir.ActivationFunctionType.Exp,
                                    scale=0.125,
                                )
                                sts.append(st_t)
                            sts_buf[kt] = sts
                        if step >= LAG:
                            kt = step - LAG
                            sts = sts_buf.pop(kt)
                            for i in range(2):
                                h = 2 * p + i
                                nc.tensor.matmul(
                                    out=avs[i][0:65, :],
                                    lhsT=v_big[:, kt, h * 65 : (h + 1) * 65],
                                    rhs=sts[i],
                                    start=(kt == 0),
                                    stop=(kt == n_k - 1),
                                )
                    # normalize: attnT = av[0:64] / Z  (Z = av row 64)
                    for i in range(2):
                        h = 2 * p + i
                        r = tmpp.tile([1, SB], f32, tag="r", name="r")
                        nc.vector.reciprocal(out=r, in_=avs[i][64:65, :])
                        zb = tmpp.tile([64, SB], f32, tag="zb", name="zb")
                        nc.gpsimd.partition_broadcast(zb, r)
                        c, half = h // 2, (h % 2) * 64
                        nc.vector.tensor_mul(
                            at8[(c, qb)][half : half + 64, :], avs[i][0:64, :], zb
                        )

            # ---- phase D: output projection (row-parallel partial) -------------
            def emit_out_proj(qb):
                for stl in range(4):
                    rows = qb * SB + stl * KT
                    for eb in range(2):
                        pw = mmp.tile([128, SB], f32, tag="mm", name="pw")
                        for c in range(2):
                            nc.tensor.matmul(
                                out=pw,
                                lhsT=at8[(c, qb)][:, stl * KT : (stl + 1) * KT],
                                rhs=wo[:, c, eb * SB : (eb + 1) * SB],
                                start=(c == 0),
                                stop=(c == 1),
                            )
                        ot = stp.tile([128, SB], f16, tag="ot", name="ot", bufs=3)
                        nc.vector.tensor_copy(out=ot, in_=pw)
                        nc.sync.dma_start(
                            out=out_d.ap()[rows : rows + KT, eb * SB : (eb + 1) * SB],
                            in_=ot,
                        )

            # ---- emission schedule: pipeline loads/proj with attention ----------
            # unroll > 1 repeats the whole kernel for overhead-free timing
            for _ in range(unroll):
                emit_loads(0)
                emit_qk_proj(0)
                emit_v_proj(0)
                emit_loads(1)
                emit_qk_proj(1)
                emit_v_proj(1)
                emit_attn(0)
                emit_loads(2)
                emit_qk_proj(2)
                emit_v_proj(2)
                emit_attn(1)
                emit_loads(3)
                emit_qk_proj(3)
                emit_v_proj(3)
                emit_out_proj(0)
                emit_attn(2)
                emit_out_proj(1)
                emit_attn(3)
                emit_out_proj(2)
                emit_out_proj(3)

    nc.compile()
    return nc


def build_in_maps(x, Wq, Wk, Wv, Wo):
    x = np.asarray(x, np.float32)
    Wq = np.asarray(Wq, np.float32)
    Wk = np.asarray(Wk, np.float32)
    Wv = np.asarray(Wv, np.float32)
    Wo = np.asarray(Wo, np.float32)

    # RoPE tables in rotate-half layout ([32] pair-frequencies, duplicated)
    inv = 1.0 / (ROPE_BASE ** (np.arange(0, D, 2, dtype=np.float64) / D))  # [32]
    ang = inv[:, None] * np.arange(S, dtype=np.float64)[None, :]  # [32, S]
    cos_t = np.cos(ang).astype(np.float32)
    sin_t = np.sin(ang).astype(np.float32)
    cos2 = np.concatenate([cos_t, cos_t, cos_t, cos_t], 0)  # [128, S]
    sin2 = np.concatenate([sin_t, -sin_t, sin_t, -sin_t], 0)  # [128, S] (rows at swap-source positions)

    # Causal mask matmul operands: accumulating tri.T @ maskb_j into the
    # scores psum adds -240*max(0, r + 128j - c), which the exp flushes to 0
    # exactly on the masked (k > q) region.
    tt = np.arange(128)[:, None]
    cc = np.arange(SB)[None, :]
    maskb = np.ascontiguousarray(
        np.stack([(cc < tt + j * KT) for j in range(4)], axis=1)
    ).astype(np.float16)  # [128, 4, SB]
    rr = np.arange(128)[None, :]
    tri = (-240.0 * (tt <= rr)).astype(np.float16)  # [t, r]

    # weight column permutation: even pair-elements then odd (rotate-half)
    perm = np.concatenate([np.arange(0, D, 2), np.arange(1, D, 2)])

    in_maps = []
    for core in range(NCORES):
        b, g = core // HPC, core % HPC
        wqk = np.empty((E, HPC, 128), np.float32)
        for i in range(HPC):
            h = g * HPC + i
            wqk[:, i, 0:64] = Wq[:, h * D : (h + 1) * D][:, perm]
            wqk[:, i, 64:128] = Wk[:, h * D : (h + 1) * D][:, perm]
        in_maps.append(
            {
                "xT": np.ascontiguousarray(x[b].T).astype(np.float16),
                "wqk": wqk.astype(np.float16),
                "wv": np.ascontiguousarray(
                    Wv[:, g * HPC * D : (g + 1) * HPC * D]
                ).astype(np.float16),
                "wo": np.ascontiguousarray(
                    Wo[g * HPC * D : (g + 1) * HPC * D, :]
                ).astype(np.float16),
                "cos2": cos2,
                "sin2": sin2,
                "maskb": maskb,
                "tri": tri,
            }
        )
    return in_maps


def gather_output(results):
    outs = [np.asarray(r["out"], np.float32) for r in results]
    return np.stack(
        [outs[0] + outs[1] + outs[2] + outs[3], outs[4] + outs[5] + outs[6] + outs[7]],
        axis=0,
    )


_NC_CACHE = {}


def kernel(x, Wq, Wk, Wv, Wo):
    in_maps = build_in_maps(x, Wq, Wk, Wv, Wo)
    if "nc" not in _NC_CACHE:
        _NC_CACHE["nc"] = build_nc()
    res = run_bass_kernel_spmd(_NC_CACHE["nc"], in_maps, core_ids=list(range(NCORES)))
    return gather_output(res.results)


# revision 17
# speedup vs baseline: 3.3031x; 1.1844x over previous
"""Multi-head causal attention (B=2, S=2048, E=1024, H=16, D=64) on 8 TRN2 cores.

Sharding: core c -> batch b = c // 4, head group g = c % 4 (4 heads each).
Each core computes q/k/v projections + RoPE + causal attention + its rows of
the Wo projection for its (batch, head-group); the host sums the 4 row-parallel
Wo partials per batch (the unshard step of row-parallel output projection).

Device layout notes:
  - x is passed pre-transposed per batch: xT [E, S] so the PE can contract
    over E (partition dim) for the projections.
  - q/k are computed transposed (qT/kT [64, S]) with head-pair fused weights
    so one [128, 512] PSUM tile holds [q_x1; q_x2; k_x1; k_x2] rows, where
    x1/x2 are the RoPE even/odd pair halves (weight columns pre-permuted on
    host so rotate-half applies).
  - scores are computed transposed, sT [k, q] = kT.T @ qT; softmax runs over
    the partition dim via an appended ones-column in the AV matmul (Z row).
    No max-subtraction: scores ~ N(0,1), exp is safe in fp32.
  - AV computes attnT [d, q]; Wo projection contracts head-dim chunks of
    attnT against Wo rows (fp16), accumulating out [s, e] tiles in PSUM.
"""

import sys

if "/opt/trn_rl_repo" not in sys.path:
    sys.path.insert(0, "/opt/trn_rl_repo")

import numpy as np

import concourse.bass as bass
import concourse.tile as tile
from concourse import bacc, mybir
from concourse.bass_utils import run_bass_kernel_spmd

B, S, E, H, D = 2, 2048, 1024, 16, 64
HPC = 4  # heads per core
NCORES = 8
SB = 512  # q/s block width
NSB = S // SB  # 4
KT = 128  # k tile (partition chunk of the sequence)
NKT = S // KT  # 16
ECH = E // 128  # 8 contraction chunks for the projections

f32 = mybir.dt.float32
f16 = mybir.dt.float16
bf16 = mybir.dt.bfloat16

ROPE_BASE = 10000.0


def build_nc(unroll=1):
    nc = bacc.Bacc(
        "TRN2", target_bir_lowering=False, debug=False, enable_asserts=False
    )

    xT_d = nc.dram_tensor("xT", [E, S], f16, kind="ExternalInput")
    wqk_d = nc.dram_tensor("wqk", [E, HPC, 128], f16, kind="ExternalInput")
    wv_d = nc.dram_tensor("wv", [E, HPC * D], f16, kind="ExternalInput")
    wo_d = nc.dram_tensor("wo", [HPC * D, E], f16, kind="ExternalInput")
    cos_d = nc.dram_tensor("cos2", [128, S], f32, kind="ExternalInput")
    sin_d = nc.dram_tensor("sin2", [128, S], f32, kind="ExternalInput")
    mask_d = nc.dram_tensor("maskb", [128, 4, SB], f16, kind="ExternalInput")
    tri_d = nc.dram_tensor("tri", [128, 128], f16, kind="ExternalInput")
    out_d = nc.dram_tensor("out", [S, E], f16, kind="ExternalOutput")

    with tile.TileContext(nc) as tc:
        with (
            tc.tile_pool(name="const", bufs=1) as constp,
            tc.tile_pool(name="qk", bufs=1) as qkp,
            tc.tile_pool(name="vb", bufs=1) as vbp,
            tc.tile_pool(name="at", bufs=1) as atp,
            tc.tile_pool(name="st", bufs=12) as stp,
            tc.tile_pool(name="tmp", bufs=4) as tmpp,
            tc.tile_pool(name="mm", bufs=6, space="PSUM") as mmp,
            tc.tile_pool(name="acc", bufs=2, space="PSUM") as accp,
        ):
            # ---- constant tiles (DMAs issued per s-block, in consumption
            # order, so the first projection matmuls start within a few us) --
            xT_ap = xT_d.ap().rearrange("(eo p) s -> eo p s", p=128)
            xT = [
                constp.tile([128, S], f16, tag=f"xT{e}", name=f"xT{e}")
                for e in range(ECH)
            ]
            wqk = constp.tile([128, ECH, HPC, 128], f16, tag="wqk")
            nc.sync.dma_start(
                out=wqk, in_=wqk_d.ap().rearrange("(eo p) h m -> p eo h m", p=128)
            )
            wv = constp.tile([128, ECH, HPC * D], f16, tag="wv")
            nc.sync.dma_start(
                out=wv, in_=wv_d.ap().rearrange("(eo p) m -> p eo m", p=128)
            )
            cos2 = constp.tile([128, S], f32, tag="cos2")
            sin2 = constp.tile([128, S], f32, tag="sin2")
            maskb = constp.tile([128, 4, SB], f16, tag="maskb")
            tri = constp.tile([128, 128], f16, tag="tri")
            wo = constp.tile([128, 2, E], f16, tag="wo")

            def emit_loads(sb):
                cs = slice(sb * SB, (sb + 1) * SB)
                for e in range(ECH):
                    nc.sync.dma_start(out=xT[e][:, cs], in_=xT_ap[e][:, cs])
                nc.sync.dma_start(out=cos2[:, cs], in_=cos_d.ap()[:, cs])
                nc.sync.dma_start(out=sin2[:, cs], in_=sin_d.ap()[:, cs])
                if sb == 0:
                    nc.sync.dma_start(out=maskb, in_=mask_d.ap())
                    nc.sync.dma_start(out=tri, in_=tri_d.ap())
                if sb == 1:
                    nc.sync.dma_start(
                        out=wo, in_=wo_d.ap().rearrange("(c p) e -> p c e", p=128)
                    )

            # qq[p] rows: qT of head 2p on partitions 0-63, head 2p+1 on 64-127
            # (kk[p] likewise) so each head's scores matmul operands share a
            # partition base. psum rows per head: [q_x1; q_x2; k_x1; k_x2].
            qq = [
                qkp.tile([128, S], f16, tag=f"qq{p}", name=f"qq{p}")
                for p in range(2)
            ]
            kk = [
                qkp.tile([128, S], f16, tag=f"kk{p}", name=f"kk{p}")
                for p in range(2)
            ]
            swap_src = [32, 0, 96, 64]

            def emit_qk_proj(sb):
                cs = slice(sb * SB, (sb + 1) * SB)
                for h in range(HPC):
                    p, half = h // 2, (h % 2) * 64
                    ps = mmp.tile([128, SB], f32, tag="mm", name="ps")
                    for e in range(ECH):
                        nc.tensor.matmul(
                            out=ps,
                            lhsT=wqk[:, e, h, :],
                            rhs=xT[e][:, cs],
                            start=(e == 0),
                            stop=(e == ECH - 1),
                        )
                    rs = tmpp.tile([128, SB], f32, tag="rs", name="rs")
                    nc.scalar.copy(out=rs, in_=ps)
                    t1 = tmpp.tile([128, SB], f32, tag="t1", name="t1")
                    t2 = tmpp.tile([128, SB], f32, tag="t2", name="t2")
                    nc.vector.tensor_mul(t1, rs, cos2[:, cs])
                    for g in range(4):
                        # sin2 rows are laid out so in0/in1 share a base
                        # partition (walrus SB+SB constraint)
                        srow = swap_src[g]
                        nc.vector.tensor_mul(
                            t2[g * 32 : (g + 1) * 32, :],
                            rs[srow : srow + 32, :],
                            sin2[srow : srow + 32, cs],
                        )
                    nc.vector.tensor_add(
                        qq[p][half : half + 64, cs], t1[0:64, :], t2[0:64, :]
                    )
                    nc.vector.tensor_add(
                        kk[p][half : half + 64, cs], t1[64:128, :], t2[64:128, :]
                    )

            # v_big free layout per k-chunk: 4 heads x [v_h (64) | one (1)]
            v_big = vbp.tile([128, NKT, HPC * 65], f16, tag="vbig")
            ones_cols = v_big.rearrange("p n (h m) -> p n h m", h=HPC)[
                :, :, :, 64:65
            ]
            nc.vector.memset(ones_cols, 1.0)

            def emit_v_proj(sb):
                for kc in range(4 * sb, 4 * sb + 4):
                    vps = mmp.tile([128, HPC * D], f32, tag="mm", name="vps")
                    for e in range(ECH):
                        nc.tensor.matmul(
                            out=vps,
                            lhsT=xT[e][:, kc * KT : (kc + 1) * KT],
                            rhs=wv[:, e, :],
                            start=(e == 0),
                            stop=(e == ECH - 1),
                        )
                    nc.vector.tensor_copy(
                        out=v_big.rearrange("p n (h m) -> p n h m", h=HPC)[
                            :, kc, :, 0:64
                        ],
                        in_=vps.rearrange("p (h m) -> p h m", h=HPC),
                    )

            # ---- phase C: attention per (q block, head pair) --------------------
            # attnT tiles: at8[c][qb] rows = hd chunk c (2 heads x 64), cols = q
            # Heads 2p / 2p+1 sit at partition bases 0 / 64 of qq[p]/kk[p], so
            # their K=64 scores matmuls land in disjoint PE row groups and run
            # concurrently (row tiling via auto tile_position).
            at8 = {}
            for c in range(2):
                for qb in range(NSB):
                    at8[(c, qb)] = atp.tile(
                        [128, SB], f16, tag=f"at{c}_{qb}", name=f"at{c}_{qb}"
                    )

            def emit_attn(qb):
                qs = slice(qb * SB, (qb + 1) * SB)
                n_k = 4 * (qb + 1)
                for p in range(2):
                    avs = [
                        accp.tile([128, SB], f32, tag="acc", name=f"av{i}")
                        for i in range(2)
                    ]
                    # Software pipeline: emit the AV matmul for chunk kt only
                    # LAG steps after its scores matmul, so the PE (strict
                    # in-order queue) never head-of-line blocks on the ACT exp.
                    LAG = 4
                    sts_buf = {}
                    for step in range(n_k + LAG):
                        if step < n_k:
                            kt = step
                            j = kt - 4 * qb
                            kts = slice(kt * KT, (kt + 1) * KT)
                            pss, sts = [], []
                            for i in range(2):
                                half = i * 64
                                ps = mmp.tile([128, SB], f32, tag="mm", name="ps")
                                nc.tensor.matmul(
                                    out=ps,
                                    lhsT=kk[p][half : half + 64, kts],
                                    rhs=qq[p][half : half + 64, qs],
                                    start=True,
                                    stop=(j < 0),
                                )
                                if j >= 0:
                                    # causal mask: add -240*max(0, r+128j-c)
                                    # (tri.T @ maskb_j); exp(0.125*x) -> 0
                                    nc.tensor.matmul(
                                        out=ps,
                                        lhsT=tri,
                                        rhs=maskb[:, j, :],
                                        start=False,
                                        stop=True,
                                    )
                                pss.append(ps)
                            for i in range(2):
                                st_t = stp.tile(
                                    [128, SB], f16, tag="st", name="st_t"
                                )
                                nc.scalar.activation(
                                    out=st_t,
                                    in_=pss[i],
                                    func=mybir.ActivationFunctionType.Exp,
                                    scale=0.125,
                                )
                                sts.append(st_t)
                            sts_buf[kt] = sts
                        if step >= LAG:
                            kt = step - LAG
                            sts = sts_buf.pop(kt)
                            for i in range(2):
                                h = 2 * p + i
                                nc.tensor.matmul(
                                    out=avs[i][0:65, :],
                                    lhsT=v_big[:, kt, h * 65 : (h + 1) * 65],
                                    rhs=sts[i],
                                    start=(kt == 0),
                                    stop=(kt == n_k - 1),
                                )
                    # normalize: attnT = av[0:64] / Z  (Z = av row 64)
                    for i in range(2):
                        h = 2 * p + i
                        r = tmpp.tile([1, SB], f32, tag="r", name="r")
                        nc.vector.reciprocal(out=r, in_=avs[i][64:65, :])
                        zb = tmpp.tile([64, SB], f32, tag="zb", name="zb")
                        nc.gpsimd.partition_broadcast(zb, r)
                        c, half = h // 2, (h % 2) * 64
                        nc.vector.tensor_mul(
                            at8[(c, qb)][half : half + 64, :], avs[i][0:64, :], zb
                        )

            # ---- phase D: output projection (row-parallel partial) -------------
            def emit_out_proj(qb):
                for stl in range(4):
                    rows = qb * SB + stl * KT
                    for eb in range(2):
                        pw = mmp.tile([128, SB], f32, tag="mm", name="pw")
                        for c in range(2):
                            nc.tensor.matmul(
                                out=pw,
                                lhsT=at8[(c, qb)][:, stl * KT : (stl + 1) * KT],
                                rhs=wo[:, c, eb * SB : (eb + 1) * SB],
                                start=(c == 0),
                                stop=(c == 1),
                            )
                        ot = stp.tile([128, SB], f16, tag="ot", name="ot", bufs=3)
                        nc.vector.tensor_copy(out=ot, in_=pw)
                        nc.sync.dma_start(
                            out=out_d.ap()[rows : rows + KT, eb * SB : (eb + 1) * SB],
                            in_=ot,
                        )

            # ---- emission schedule: pipeline loads/proj with attention ----------
            # unroll > 1 repeats the whole kernel for overhead-free timing
            for _ in range(unroll):
                emit_loads(0)
                emit_qk_proj(0)
                emit_v_proj(0)
                emit_loads(1)
                emit_qk_proj(1)
                emit_v_proj(1)
                emit_attn(0)
                emit_loads(2)
                emit_qk_proj(2)
                emit_v_proj(2)
                emit_attn(1)
                emit_loads(3)
                emit_qk_proj(3)
                emit_v_proj(3)
                emit_out_proj(0)
                emit_attn(2)
                emit_out_proj(1)
                emit_attn(3)
                emit_out_proj(2)
                emit_out_proj(3)

    nc.compile()
    return nc


def build_in_maps(x, Wq, Wk, Wv, Wo):
    x = np.asarray(x, np.float32)
    Wq = np.asarray(Wq, np.float32)
    Wk = np.asarray(Wk, np.float32)
    Wv = np.asarray(Wv, np.float32)
    Wo = np.asarray(Wo, np.float32)

    # RoPE tables in rotate-half layout ([32] pair-frequencies, duplicated)
    inv = 1.0 / (ROPE_BASE ** (np.arange(0, D, 2, dtype=np.float64) / D))  # [32]
    ang = inv[:, None] * np.arange(S, dtype=np.float64)[None, :]  # [32, S]
    cos_t = np.cos(ang).astype(np.float32)
    sin_t = np.sin(ang).astype(np.float32)
    cos2 = np.concatenate([cos_t, cos_t, cos_t, cos_t], 0)  # [128, S]
    sin2 = np.concatenate([sin_t, -sin_t, sin_t, -sin_t], 0)  # [128, S] (rows at swap-source positions)

    # Causal mask matmul operands: accumulating tri.T @ maskb_j into the
    # scores psum adds -240*max(0, r + 128j - c), which the exp flushes to 0
    # exactly on the masked (k > q) region.
    tt = np.arange(128)[:, None]
    cc = np.arange(SB)[None, :]
    maskb = np.ascontiguousarray(
        np.stack([(cc < tt + j * KT) for j in range(4)], axis=1)
    ).astype(np.float16)  # [128, 4, SB]
    rr = np.arange(128)[None, :]
    tri = (-240.0 * (tt <= rr)).astype(np.float16)  # [t, r]

    # weight column permutation: even pair-elements then odd (rotate-half)
    perm = np.concatenate([np.arange(0, D, 2), np.arange(1, D, 2)])

    in_maps = []
    for core in range(NCORES):
        b, g = core // HPC, core % HPC
        wqk = np.empty((E, HPC, 128), np.float32)
        for i in range(HPC):
            h = g * HPC + i
            wqk[:, i, 0:64] = Wq[:, h * D : (h + 1) * D][:, perm]
            wqk[:, i, 64:128] = Wk[:, h * D : (h + 1) * D][:, perm]
        in_maps.append(
            {
                "xT": np.ascontiguousarray(x[b].T).astype(np.float16),
                "wqk": wqk.astype(np.float16),
                "wv": np.ascontiguousarray(
                    Wv[:, g * HPC * D : (g + 1) * HPC * D]
                ).astype(np.float16),
                "wo": np.ascontiguousarray(
                    Wo[g * HPC * D : (g + 1) * HPC * D, :]
                ).astype(np.float16),
                "cos2": cos2,
                "sin2": sin2,
                "maskb": maskb,
                "tri": tri,
            }
        )
    return in_maps


def gather_output(results):
    outs = [np.asarray(r["out"], np.float32) for r in results]
    return np.stack(
        [outs[0] + outs[1] + outs[2] + outs[3], outs[4] + outs[5] + outs[6] + outs[7]],
        axis=0,
    )


_NC_CACHE = {}


def kernel(x, Wq, Wk, Wv, Wo):
    in_maps = build_in_maps(x, Wq, Wk, Wv, Wo)
    if "nc" not in _NC_CACHE:
        _NC_CACHE["nc"] = build_nc()
    res = run_bass_kernel_spmd(_NC_CACHE["nc"], in_maps, core_ids=list(range(NCORES)))
    return gather_output(res.results)


# revision 19
# speedup vs baseline: 60.2579x; 18.2429x over previous
"""Multi-head causal attention (B=2, S=2048, E=1024, H=16, D=64) on 8 TRN2 cores.

Sharding: core c -> batch b = c // 4, head group g = c % 4 (4 heads each).
Each core computes q/k/v projections + RoPE + causal attention + its rows of
the Wo projection for its (batch, head-group); the host sums the 4 row-parallel
Wo partials per batch (the unshard step of row-parallel output projection).

Device layout notes:
  - x is passed pre-transposed per batch: xT [E, S] so the PE can contract
    over E (partition dim) for the projections.
  - q/k are computed transposed (qT/kT [64, S]) with head-pair fused weights
    so one [128, 512] PSUM tile holds [q_x1; q_x2; k_x1; k_x2] rows, where
    x1/x2 are the RoPE even/odd pair halves (weight columns pre-permuted on
    host so rotate-half applies).
  - scores are computed transposed, sT [k, q] = kT.T @ qT; softmax runs over
    the partition dim via an appended ones-column in the AV matmul (Z row).
    No max-subtraction: scores ~ N(0,1), exp is safe in fp32.
  - AV computes attnT [d, q]; Wo projection contracts head-dim chunks of
    attnT against Wo rows (fp16), accumulating out [s, e] tiles in PSUM.
"""

import sys

if "/opt/trn_rl_repo" not in sys.path:
    sys.path.insert(0, "/opt/trn_rl_repo")

import numpy as np

import concourse.bass as bass
import concourse.tile as tile
from concourse import bacc, mybir
from concourse.bass_utils import run_bass_kernel_spmd

B, S, E, H, D = 2, 2048, 1024, 16, 64
HPC = 4  # heads per core
NCORES = 8
SB = 512  # q/s block width
NSB = S // SB  # 4
KT = 128  # k tile (partition chunk of the sequence)
NKT = S // KT  # 16
ECH = E // 128  # 8 contraction chunks for the projections

f32 = mybir.dt.float32
f16 = mybir.dt.float16
bf16 = mybir.dt.bfloat16

ROPE_BASE = 10000.0


def build_nc(unroll=1):
    nc = bacc.Bacc(
        "TRN2", target_bir_lowering=False, debug=False, enable_asserts=False
    )

    xT_d = nc.dram_tensor("xT", [E, S], f16, kind="ExternalInput")
    wqk_d = nc.dram_tensor("wqk", [E, HPC, 128], f16, kind="ExternalInput")
    wv_d = nc.dram_tensor("wv", [E, HPC * D], f16, kind="ExternalInput")
    wo_d = nc.dram_tensor("wo", [HPC * D, E], f16, kind="ExternalInput")
    cos_d = nc.dram_tensor("cos2", [128, S], f32, kind="ExternalInput")
    sin_d = nc.dram_tensor("sin2", [128, S], f32, kind="ExternalInput")
    mask_d = nc.dram_tensor("maskb", [128, 4, SB], f16, kind="ExternalInput")
    tri_d = nc.dram_tensor("tri", [128, 128], f16, kind="ExternalInput")
    out_d = nc.dram_tensor("out", [S, E], f16, kind="ExternalOutput")

    with tile.TileContext(nc) as tc:
        with (
            tc.tile_pool(name="const", bufs=1) as constp,
            tc.tile_pool(name="qk", bufs=1) as qkp,
            tc.tile_pool(name="vb", bufs=1) as vbp,
            tc.tile_pool(name="at", bufs=1) as atp,
            tc.tile_pool(name="st", bufs=12) as stp,
            tc.tile_pool(name="tmp", bufs=4) as tmpp,
            tc.tile_pool(name="mm", bufs=2, space="PSUM") as mmp,
            tc.tile_pool(name="wps", bufs=2, space="PSUM") as wpsp,
            tc.tile_pool(name="acc", bufs=1, space="PSUM") as accp,
        ):
            # ---- constant tiles (DMAs issued per s-block, in consumption
            # order, so the first projection matmuls start within a few us) --
            xT_ap = xT_d.ap().rearrange("(eo p) s -> eo p s", p=128)
            xT = [
                constp.tile([128, S], f16, tag=f"xT{e}", name=f"xT{e}")
                for e in range(ECH)
            ]
            wqk = constp.tile([128, ECH, HPC, 128], f16, tag="wqk")
            nc.sync.dma_start(
                out=wqk, in_=wqk_d.ap().rearrange("(eo p) h m -> p eo h m", p=128)
            )
            wv = constp.tile([128, ECH, HPC * D], f16, tag="wv")
            nc.sync.dma_start(
                out=wv, in_=wv_d.ap().rearrange("(eo p) m -> p eo m", p=128)
            )
            cos2 = constp.tile([128, S], f32, tag="cos2")
            sin2 = constp.tile([128, S], f32, tag="sin2")
            maskb = constp.tile([128, 4, SB], f16, tag="maskb")
            tri = constp.tile([128, 128], f16, tag="tri")
            wo = constp.tile([128, 2, E], f16, tag="wo")

            def emit_loads(sb):
                cs = slice(sb * SB, (sb + 1) * SB)
                for e in range(ECH):
                    nc.sync.dma_start(out=xT[e][:, cs], in_=xT_ap[e][:, cs])
                nc.sync.dma_start(out=cos2[:, cs], in_=cos_d.ap()[:, cs])
                nc.sync.dma_start(out=sin2[:, cs], in_=sin_d.ap()[:, cs])
                if sb == 0:
                    nc.sync.dma_start(out=maskb, in_=mask_d.ap())
                    nc.sync.dma_start(out=tri, in_=tri_d.ap())
                if sb == 1:
                    nc.sync.dma_start(
                        out=wo, in_=wo_d.ap().rearrange("(c p) e -> p c e", p=128)
                    )

            # qq[p] rows: qT of head 2p on partitions 0-63, head 2p+1 on 64-127
            # (kk[p] likewise) so each head's scores matmul operands share a
            # partition base. psum rows per head: [q_x1; q_x2; k_x1; k_x2].
            qq = [
                qkp.tile([128, S], f16, tag=f"qq{p}", name=f"qq{p}")
                for p in range(2)
            ]
            kk = [
                qkp.tile([128, S], f16, tag=f"kk{p}", name=f"kk{p}")
                for p in range(2)
            ]
            swap_src = [32, 0, 96, 64]

            def emit_qk_proj(sb):
                cs = slice(sb * SB, (sb + 1) * SB)
                for h in range(HPC):
                    p, half = h // 2, (h % 2) * 64
                    ps = mmp.tile([128, SB], f32, tag="mm", name="ps")
                    for e in range(ECH):
                        nc.tensor.matmul(
                            out=ps,
                            lhsT=wqk[:, e, h, :],
                            rhs=xT[e][:, cs],
                            start=(e == 0),
                            stop=(e == ECH - 1),
                        )
                    rs = tmpp.tile([128, SB], f32, tag="rs", name="rs")
                    nc.scalar.copy(out=rs, in_=ps)
                    t1 = tmpp.tile([128, SB], f32, tag="t1", name="t1")
                    t2 = tmpp.tile([128, SB], f32, tag="t2", name="t2")
                    nc.vector.tensor_mul(t1, rs, cos2[:, cs])
                    for g in range(4):
                        # sin2 rows are laid out so in0/in1 share a base
                        # partition (walrus SB+SB constraint)
                        srow = swap_src[g]
                        nc.vector.tensor_mul(
                            t2[g * 32 : (g + 1) * 32, :],
                            rs[srow : srow + 32, :],
                            sin2[srow : srow + 32, cs],
                        )
                    nc.vector.tensor_add(
                        qq[p][half : half + 64, cs], t1[0:64, :], t2[0:64, :]
                    )
                    nc.vector.tensor_add(
                        kk[p][half : half + 64, cs], t1[64:128, :], t2[64:128, :]
                    )

            # v_big free layout per k-chunk: 4 heads x [v_h (64) | one (1)]
            v_big = vbp.tile([128, NKT, HPC * 65], f16, tag="vbig")
            ones_cols = v_big.rearrange("p n (h m) -> p n h m", h=HPC)[
                :, :, :, 64:65
            ]
            nc.vector.memset(ones_cols, 1.0)

            def emit_v_proj(sb):
                for kc in range(4 * sb, 4 * sb + 4):
                    vps = mmp.tile([128, HPC * D], f32, tag="mm", name="vps")
                    for e in range(ECH):
                        nc.tensor.matmul(
                            out=vps,
                            lhsT=xT[e][:, kc * KT : (kc + 1) * KT],
                            rhs=wv[:, e, :],
                            start=(e == 0),
                            stop=(e == ECH - 1),
                        )
                    nc.vector.tensor_copy(
                        out=v_big.rearrange("p n (h m) -> p n h m", h=HPC)[
                            :, kc, :, 0:64
                        ],
                        in_=vps.rearrange("p (h m) -> p h m", h=HPC),
                    )

            # ---- phase C: attention per (q block, head pair) --------------------
            # attnT tiles: at8[c][qb] rows = hd chunk c (2 heads x 64), cols = q
            # Heads 2p / 2p+1 sit at partition bases 0 / 64 of qq[p]/kk[p], so
            # their K=64 scores matmuls land in disjoint PE row groups and run
            # concurrently (row tiling via auto tile_position).
            at8 = {}
            for c in range(2):
                for qb in range(NSB):
                    at8[(c, qb)] = atp.tile(
                        [128, SB], f16, tag=f"at{c}_{qb}", name=f"at{c}_{qb}"
                    )

            def emit_attn(qb):
                qs = slice(qb * SB, (qb + 1) * SB)
                n_k = 4 * (qb + 1)
                for p in range(2):
                    # one wide [128, 1024] PSUM pair-tile per head pair: both
                    # heads' scores live side by side so a single ACT exp
                    # covers them (halves exp instructions and sem hops)
                    av2 = accp.tile([128, 2 * SB], f32, tag="acc", name="av2")
                    # Software pipeline: emit the AV matmul for chunk kt only
                    # LAG steps after its scores matmul, so the PE (strict
                    # in-order queue) never head-of-line blocks on the ACT exp.
                    LAG = 2
                    sts_buf = {}
                    for step in range(n_k + LAG):
                        if step < n_k:
                            kt = step
                            j = kt - 4 * qb
                            kts = slice(kt * KT, (kt + 1) * KT)
                            ps2 = wpsp.tile(
                                [128, 2 * SB], f32, tag="wps", name="ps2"
                            )
                            for i in range(2):
                                half = i * 64
                                nc.tensor.matmul(
                                    out=ps2[:, i * SB : (i + 1) * SB],
                                    lhsT=kk[p][half : half + 64, kts],
                                    rhs=qq[p][half : half + 64, qs],
                                    start=True,
                                    stop=(j < 0),
                                )
                                if j >= 0:
                                    # causal mask: add -240*max(0, r+128j-c)
                                    # (tri.T @ maskb_j); exp(0.125*x) -> 0
                                    nc.tensor.matmul(
                                        out=ps2[:, i * SB : (i + 1) * SB],
                                        lhsT=tri,
                                        rhs=maskb[:, j, :],
                                        start=False,
                                        stop=True,
                                    )
                            st_t = stp.tile(
                                [128, 2 * SB], f16, tag="st", name="st_t"
                            )
                            nc.scalar.activation(
                                out=st_t,
                                in_=ps2,
                                func=mybir.ActivationFunctionType.Exp,
                                scale=0.125,
                            )
                            sts_buf[kt] = st_t
                        if step >= LAG:
                            kt = step - LAG
                            st_t = sts_buf.pop(kt)
                            for i in range(2):
                                h = 2 * p + i
                                nc.tensor.matmul(
                                    out=av2[0:65, i * SB : (i + 1) * SB],
                                    lhsT=v_big[:, kt, h * 65 : (h + 1) * 65],
                                    rhs=st_t[:, i * SB : (i + 1) * SB],
                                    start=(kt == 0),
                                    stop=(kt == n_k - 1),
                                )
                    # normalize: attnT = av[0:64] / Z  (Z = av row 64)
                    for i in range(2):
                        h = 2 * p + i
                        avi = av2[:, i * SB : (i + 1) * SB]
                        r = tmpp.tile([1, SB], f32, tag="r", name="r")
                        nc.vector.reciprocal(out=r, in_=avi[64:65, :])
                        zb = tmpp.tile([64, SB], f32, tag="zb", name="zb")
                        nc.gpsimd.partition_broadcast(zb, r)
                        c, half = h // 2, (h % 2) * 64
                        nc.vector.tensor_mul(
                            at8[(c, qb)][half : half + 64, :], avi[0:64, :], zb
                        )

            # ---- phase D: output projection (row-parallel partial) -------------
            def emit_out_proj(qb):
                for stl in range(4):
                    rows = qb * SB + stl * KT
                    for eb in range(2):
                        pw = mmp.tile([128, SB], f32, tag="mm", name="pw")
                        for c in range(2):
                            nc.tensor.matmul(
                                out=pw,
                                lhsT=at8[(c, qb)][:, stl * KT : (stl + 1) * KT],
                                rhs=wo[:, c, eb * SB : (eb + 1) * SB],
                                start=(c == 0),
                                stop=(c == 1),
                            )
                        ot = stp.tile([128, SB], f16, tag="ot", name="ot", bufs=3)
                        nc.vector.tensor_copy(out=ot, in_=pw)
                        nc.sync.dma_start(
                            out=out_d.ap()[rows : rows + KT, eb * SB : (eb + 1) * SB],
                            in_=ot,
                        )

            # ---- emission schedule: pipeline loads/proj with attention ----------
            # unroll > 1 repeats the whole kernel for overhead-free timing
            for _ in range(unroll):
                emit_loads(0)
                emit_qk_proj(0)
                emit_v_proj(0)
                emit_loads(1)
                emit_qk_proj(1)
                emit_v_proj(1)
                emit_attn(0)
                emit_loads(2)
                emit_qk_proj(2)
                emit_v_proj(2)
                emit_attn(1)
                emit_loads(3)
                emit_qk_proj(3)
                emit_v_proj(3)
                emit_out_proj(0)
                emit_attn(2)
                emit_out_proj(1)
                emit_attn(3)
                emit_out_proj(2)
                emit_out_proj(3)

    nc.compile()
    return nc


def build_in_maps(x, Wq, Wk, Wv, Wo):
    x = np.asarray(x, np.float32)
    Wq = np.asarray(Wq, np.float32)
    Wk = np.asarray(Wk, np.float32)
    Wv = np.asarray(Wv, np.float32)
    Wo = np.asarray(Wo, np.float32)

    # RoPE tables in rotate-half layout ([32] pair-frequencies, duplicated)
    inv = 1.0 / (ROPE_BASE ** (np.arange(0, D, 2, dtype=np.float64) / D))  # [32]
    ang = inv[:, None] * np.arange(S, dtype=np.float64)[None, :]  # [32, S]
    cos_t = np.cos(ang).astype(np.float32)
    sin_t = np.sin(ang).astype(np.float32)
    cos2 = np.concatenate([cos_t, cos_t, cos_t, cos_t], 0)  # [128, S]
    sin2 = np.concatenate([sin_t, -sin_t, sin_t, -sin_t], 0)  # [128, S] (rows at swap-source positions)

    # Causal mask matmul operands: accumulating tri.T @ maskb_j into the
    # scores psum adds -240*max(0, r + 128j - c), which the exp flushes to 0
    # exactly on the masked (k > q) region.
    tt = np.arange(128)[:, None]
    cc = np.arange(SB)[None, :]
    maskb = np.ascontiguousarray(
        np.stack([(cc < tt + j * KT) for j in range(4)], axis=1)
    ).astype(np.float16)  # [128, 4, SB]
    rr = np.arange(128)[None, :]
    tri = (-240.0 * (tt <= rr)).astype(np.float16)  # [t, r]

    # weight column permutation: even pair-elements then odd (rotate-half)
    perm = np.concatenate([np.arange(0, D, 2), np.arange(1, D, 2)])

    in_maps = []
    for core in range(NCORES):
        b, g = core // HPC, core % HPC
        wqk = np.empty((E, HPC, 128), np.float32)
        for i in range(HPC):
            h = g * HPC + i
            wqk[:, i, 0:64] = Wq[:, h * D : (h + 1) * D][:, perm]
            wqk[:, i, 64:128] = Wk[:, h * D : (h + 1) * D][:, perm]
        in_maps.append(
            {
                "xT": np.ascontiguousarray(x[b].T).astype(np.float16),
                "wqk": wqk.astype(np.float16),
                "wv": np.ascontiguousarray(
                    Wv[:, g * HPC * D : (g + 1) * HPC * D]
                ).astype(np.float16),
                "wo": np.ascontiguousarray(
                    Wo[g * HPC * D : (g + 1) * HPC * D, :]
                ).astype(np.float16),
                "cos2": cos2,
                "sin2": sin2,
                "maskb": maskb,
                "tri": tri,
            }
        )
    return in_maps


def gather_output(results):
    outs = [np.asarray(r["out"], np.float32) for r in results]
    return np.stack(
        [outs[0] + outs[1] + outs[2] + outs[3], outs[4] + outs[5] + outs[6] + outs[7]],
        axis=0,
    )


_NC_CACHE = {}


def kernel(x, Wq, Wk, Wv, Wo):
    in_maps = build_in_maps(x, Wq, Wk, Wv, Wo)
    if "nc" not in _NC_CACHE:
        _NC_CACHE["nc"] = build_nc()
    res = run_bass_kernel_spmd(_NC_CACHE["nc"], in_maps, core_ids=list(range(NCORES)))
    return gather_output(res.results)
